# revision 1
# baseline (speedup 1.0000x reference)
"""Trainium2 Bass kernel for nn_BertWordPair (sparse_attention).

Computes: y = x @ W1 + b1 -> split into (q_tok, q_utt, k_tok, k_utt) per
channel c in [0,3); RoPE with block-sign structure from seg_ids; output
logits [B, S, S, 3] = sum over the two groups of the selected-variant
bilinear forms.

Strategy (8 NeuronCores):
  - Data-parallel over batch (2) x query-row quarters (4): each core owns
    512 output rows of one batch and all 2048 columns.
  - Everything on device runs in transposed layout (features on
    partitions): y^T tiles come straight out of the PE with the feature
    (RoPE) dim on partitions, ready to be the contraction dim of the
    logits matmul.  x^T, the W1 column permutation, and the RoPE tables
    are precomputed on the host in partition-major packed layouts so
    each logical load is one DMA.
  - RoPE rotate-half never crosses partitions: the host permutes W1
    columns so even-d and odd-d features live in separate 128-partition
    tiles (pair (2k, 2k+1) sits at partition k of the even/odd tile).
    rope(v)_e = v_e cos - v_o sin, rope(v)_o = v_o cos + v_e sin become
    four fused (bias+table) DVE ops + an add/sub on GPSIMD.
  - The per-(row-seg, col-seg) variant selection (pp / q_neg.k_pos /
    q_pos.k_neg) reduces to two signs: sigma_q (per column block, a
    per-partition scalar when forming Q_eff) and sigma_k (per column,
    folded into the host-built SIN table for K).  Matmuls run in
    float32r (full-rate fp32, moving dim 512 >= 256).
"""
import sys
sys.path.insert(0, '/opt/trn_rl_repo')

import numpy as np

B, S, H, C = 2, 2048, 768, 3
DG = 256             # rope dim per group (tok / utt)
D2 = 512             # feature dim per channel (tok 256 + utt 256)
N_CORES = 8
QUARTERS = 4
RPC = S // QUARTERS  # 512 rows per core
BLK = 512            # column block
NB = S // BLK        # 4
KH = H // 128        # 6 contraction tiles for dense1
FT = (C * D2) // 128  # 12 feature tiles per side (q or k)
MT = RPC // 128      # 4 row tiles per core
DT = D2 // 128       # 4 d-tiles per channel: tok-e, tok-o, utt-e, utt-o


def _variant(s, t):
    # 0=PP, 1=NP (q_neg*k_pos), 2=PN (q_pos*k_neg)
    if s >= 1 and t > s:
        return 1
    if t >= 1 and s > t:
        return 2
    return 0


def _rope_tables_half(pos, base):
    """pos: [n] ints -> cos [128, n], sin [128, n]; row k = freq k."""
    freq = np.power(float(base), -2.0 * np.arange(DG // 2, dtype=np.float64) / DG)
    ang = freq[:, None] * pos[None, :].astype(np.float64)
    return np.cos(ang).astype(np.float32), np.sin(ang).astype(np.float32)


def _perm_cols(side_off):
    """New feature order: c*512 + g*256 + p*128 + k  <-  orig
    c*1024 + side_off + g*256 + 2k + p."""
    cols = np.empty(C * D2, np.int64)
    f = 0
    for c in range(C):
        for g in range(2):
            for p in range(2):
                base = c * 1024 + side_off + g * 256 + p
                cols[f:f + 128] = base + 2 * np.arange(128)
                f += 128
    return cols


def _pack_pmajor(a, nt):
    """[nt*128, F] -> [128, nt, F] (partition-major chunks)."""
    F = a.shape[1]
    return np.ascontiguousarray(a.reshape(nt, 128, F).transpose(1, 0, 2))


def _host_prep(x, W1, b1, token_index, utterance_index, seg_ids):
    """Build per-core input maps + check fast-path validity."""
    x = np.asarray(x, np.float32)
    W1 = np.asarray(W1, np.float32)
    b1 = np.asarray(b1, np.float32)
    token_index = np.asarray(token_index)
    utterance_index = np.asarray(utterance_index)
    seg_ids = np.asarray(seg_ids)

    qcols = _perm_cols(0)     # q_tok at +0, q_utt at +256
    kcols = _perm_cols(512)   # k_tok at +512, k_utt at +768
    WQp = _pack_pmajor(np.ascontiguousarray(W1[:, qcols]), KH)  # [128, KH, 1536]
    WKp = _pack_pmajor(np.ascontiguousarray(W1[:, kcols]), KH)
    bQ = b1[qcols].astype(np.float32)
    bK = b1[kcols].astype(np.float32)
    biasc = np.ascontiguousarray(
        np.concatenate([bQ, bK]).reshape(2 * FT, 128).T)  # [128, 24]

    xT = x.transpose(0, 2, 1)
    xTp = [_pack_pmajor(np.ascontiguousarray(xT[b]), KH) for b in range(B)]

    in_maps = []
    metas = []
    for core in range(N_CORES):
        b, qt = core // QUARTERS, core % QUARTERS
        rows = slice(qt * RPC, (qt + 1) * RPC)
        seg = seg_ids[b]
        s_vals = seg[rows]
        if not np.all(s_vals == s_vals[0]):
            raise NotImplementedError("fast path: core rows must share one seg")
        s = int(s_vals[0])

        var = np.array([_variant(s, int(t)) for t in seg], np.int32)
        sigq_col = np.where(var == 1, -1.0, 1.0).astype(np.float32)
        sigk_col = np.where(var == 2, -1.0, 1.0).astype(np.float32)
        sigq_blk = np.empty(NB, np.float32)
        for nb in range(NB):
            blk = sigq_col[nb * BLK:(nb + 1) * BLK]
            if not np.all(blk == blk[0]):
                raise NotImplementedError("fast path: sigma_q must be block-uniform")
            sigq_blk[nb] = blk[0]

        ct_q, st_q = _rope_tables_half(token_index[b, rows], 10000.0)
        cu_q, su_q = _rope_tables_half(utterance_index[b, rows], 15.0)
        ct_k, st_k = _rope_tables_half(token_index[b], 10000.0)
        cu_k, su_k = _rope_tables_half(utterance_index[b], 15.0)
        # [128, 2(cos/sin), 2(tok/utt), n]
        TABQ = np.ascontiguousarray(np.stack(
            [np.stack([ct_q, cu_q], 0), np.stack([st_q, su_q], 0)], 0
        ).transpose(2, 0, 1, 3))
        TABK = np.ascontiguousarray(np.stack(
            [np.stack([ct_k, cu_k], 0),
             np.stack([st_k * sigk_col[None, :], su_k * sigk_col[None, :]], 0)], 0
        ).transpose(2, 0, 1, 3))
        # [128, 2, NB]: [:,0]=-sigma_q (even), [:,1]=+sigma_q (odd)
        SIGC = np.ascontiguousarray(np.broadcast_to(
            np.stack([-sigq_blk, sigq_blk], 0)[None, :, :], (128, 2, NB)).copy())

        in_maps.append({
            "XT": xTp[b],
            "XQ": np.ascontiguousarray(xTp[b][:, :, rows]),
            "WQ": WQp, "WK": WKp, "BIASC": biasc,
            "TABQ": TABQ, "TABK": TABK, "SIGC": SIGC,
        })
        metas.append({"b": b, "qt": qt})
    return in_maps, metas


def _build_program(reps=0):
    """Build the SPMD-uniform Bass program."""
    import concourse.bacc as bacc
    import concourse.mybir as mybir
    import concourse.tile as tile
    from contextlib import ExitStack

    f32 = mybir.dt.float32
    f32r = mybir.dt.float32r
    AF = mybir.ActivationFunctionType
    OP = mybir.AluOpType

    nc = bacc.Bacc("TRN2", target_bir_lowering=False, debug=False,
                   num_devices=N_CORES)
    XT = nc.dram_tensor("XT", [128, KH, S], f32r, kind="ExternalInput")
    XQd = nc.dram_tensor("XQ", [128, KH, RPC], f32r, kind="ExternalInput")
    WQd = nc.dram_tensor("WQ", [128, KH, C * D2], f32r, kind="ExternalInput")
    WKd = nc.dram_tensor("WK", [128, KH, C * D2], f32r, kind="ExternalInput")
    BIASC = nc.dram_tensor("BIASC", [128, 2 * FT], f32, kind="ExternalInput")
    TABQ = nc.dram_tensor("TABQ", [128, 2, 2, RPC], f32, kind="ExternalInput")
    TABK = nc.dram_tensor("TABK", [128, 2, 2, S], f32, kind="ExternalInput")
    SIGC = nc.dram_tensor("SIGC", [128, 2, NB], f32, kind="ExternalInput")
    OUT = nc.dram_tensor("OUT", [C, RPC, S], f32, kind="ExternalOutput")

    with tile.TileContext(nc) as tc, ExitStack() as ctx:
        wp = ctx.enter_context(tc.tile_pool(name="wp", bufs=4))
        xp = ctx.enter_context(tc.tile_pool(name="xp", bufs=3))
        tabp = ctx.enter_context(tc.tile_pool(name="tabp", bufs=2))
        biasp = ctx.enter_context(tc.tile_pool(name="biasp", bufs=1))
        aqp = ctx.enter_context(tc.tile_pool(name="aqp", bufs=6))
        qeffp = ctx.enter_context(tc.tile_pool(name="qeffp", bufs=6))
        keffp = ctx.enter_context(tc.tile_pool(name="keffp", bufs=8))
        outp = ctx.enter_context(tc.tile_pool(name="outp", bufs=3))
        pap = ctx.enter_context(tc.tile_pool(name="pap", bufs=5, space="PSUM"))
        pbp = ctx.enter_context(tc.tile_pool(name="pbp", bufs=3, space="PSUM"))

        bias_all = biasp.tile([128, 2 * FT], f32, name="bias_all")
        nc.sync.dma_start(bias_all[:], BIASC[:])
        sig_all = biasp.tile([128, 2, NB], f32, name="sig_all")
        nc.sync.dma_start(sig_all[:], SIGC[:])

        mm = nc.tensor.matmul

        def stage_a(w_parts, xtile, ft, psum):
            third, fo = divmod(ft, FT // 3)
            for kh in range(KH):
                mm(psum[:],
                   w_parts[third][:, kh, fo * 128:(fo + 1) * 128],
                   xtile[:, kh, :],
                   start=(kh == 0), stop=(kh == KH - 1))

        def rope_pair(ps_e, ps_o, fe, tab, g, n, pool, ab_bufs=None):
            """Four fused (bias+table) products for a parity pair.
            rope_pos_e = ae - as_ ; rope_pos_o = ao + bo
            rope_neg_e = ae + as_ ; rope_neg_o = ao - bo"""
            cos = tab[:, 0, g, :]
            sin = tab[:, 1, g, :]
            be = bias_all[:, fe:fe + 1]
            bod = bias_all[:, fe + 1:fe + 2]
            ae = pool.tile([128, n], f32, name="ae", tag="ae", bufs=ab_bufs)
            bo = pool.tile([128, n], f32, name="bo", tag="bo", bufs=ab_bufs)
            as_ = pool.tile([128, n], f32, name="as_", tag="as_", bufs=ab_bufs)
            ao = pool.tile([128, n], f32, name="ao", tag="ao", bufs=ab_bufs)
            nc.vector.scalar_tensor_tensor(ae[:], ps_e[:], be, cos, OP.add, OP.mult)
            nc.vector.scalar_tensor_tensor(bo[:], ps_e[:], be, sin, OP.add, OP.mult)
            nc.vector.scalar_tensor_tensor(as_[:], ps_o[:], bod, sin, OP.add, OP.mult)
            nc.vector.scalar_tensor_tensor(ao[:], ps_o[:], bod, cos, OP.add, OP.mult)
            return ae, as_, ao, bo

        def emit_body():
            # ---------- phase Q ----------
            xq = xp.tile([128, KH, RPC], f32r, name="xq", tag="xb")
            wq_t = [wp.tile([128, KH, (C * D2) // 3], f32r, name="wt", tag="wt")
                    for _ in range(3)]
            for kh in range(KH):
                nc.sync.dma_start(xq[:, kh, :], XQd[:, kh, :])
                for third in range(3):
                    nc.sync.dma_start(
                        wq_t[third][:, kh, :],
                        WQd[:, kh, third * 512:(third + 1) * 512])
            tabq = tabp.tile([128, 2, 2, RPC], f32, name="tabq", tag="tab")
            nc.sync.dma_start(tabq[:], TABQ[:])

            ab_q = []  # per pair: (ae, as_, ao, bo)
            for pr in range(FT // 2):
                ps_e = pap.tile([128, RPC], f32, name="psa")
                stage_a(wq_t, xq, 2 * pr, ps_e)
                ps_o = pap.tile([128, RPC], f32, name="psa")
                stage_a(wq_t, xq, 2 * pr + 1, ps_o)
                ab_q.append(rope_pair(ps_e, ps_o, 2 * pr, tabq, pr % 2, RPC, aqp))

            # prefetch block 0 operands BEFORE the 4.7MB of WK loads so
            # stage-A of block 0 can start the moment WK third 1 lands
            xb0 = xp.tile([128, KH, BLK], f32r, name="xb", tag="xb")
            nc.sync.dma_start(xb0[:], XT[:, :, 0:BLK])
            tabk0 = tabp.tile([128, 2, 2, BLK], f32, name="tabk", tag="tab")
            nc.sync.dma_start(tabk0[:], TABK[:, :, :, 0:BLK])

            wk_t = []
            for third in range(3):
                wt = wp.tile([128, KH, (C * D2) // 3], f32r, name="wt", tag="wt")
                nc.sync.dma_start(wt[:], WKd[:, :, third * 512:(third + 1) * 512])
                wk_t.append(wt)

            # ---------- per column block ----------
            for nb in range(NB):
                cols = slice(nb * BLK, (nb + 1) * BLK)
                sig_e = sig_all[:, 0, nb:nb + 1]
                sig_o = sig_all[:, 1, nb:nb + 1]
                if nb == 0:
                    xb, tabk = xb0, tabk0
                else:
                    xb = xp.tile([128, KH, BLK], f32r, name="xb", tag="xb")
                    nc.sync.dma_start(xb[:], XT[:, :, cols])
                    tabk = tabp.tile([128, 2, 2, BLK], f32, name="tabk", tag="tab")
                    nc.sync.dma_start(tabk[:], TABK[:, :, :, cols])

                def emit_a(c):
                    keff_c = []
                    for g in range(2):           # tok pair, utt pair
                        ft_e = c * DT + 2 * g
                        ps_e = pap.tile([128, BLK], f32, name="psa")
                        stage_a(wk_t, xb, ft_e, ps_e)
                        ps_o = pap.tile([128, BLK], f32, name="psa")
                        stage_a(wk_t, xb, ft_e + 1, ps_o)
                        ae, as_, ao, bo = rope_pair(
                            ps_e, ps_o, FT + ft_e, tabk, g, BLK, keffp, ab_bufs=3)
                        ke_e = keffp.tile([128, BLK], f32r, name="ke", tag="ke")
                        nc.gpsimd.tensor_sub(ke_e[:], ae[:], as_[:])
                        ke_o = keffp.tile([128, BLK], f32r, name="ke", tag="ke")
                        nc.gpsimd.tensor_add(ke_o[:], ao[:], bo[:])
                        keff_c += [ke_e, ke_o]
                    return keff_c

                def emit_b(c, keff_c):
                    qeff_c = []
                    for g in range(2):
                        ae, as_, ao, bo = ab_q[c * 2 + g]
                        qe_e = qeffp.tile([128, RPC], f32r, name="qe", tag="qe")
                        nc.vector.scalar_tensor_tensor(
                            qe_e[:], as_[:], sig_e, ae[:], OP.mult, OP.add)
                        qe_o = qeffp.tile([128, RPC], f32r, name="qe", tag="qe")
                        nc.vector.scalar_tensor_tensor(
                            qe_o[:], bo[:], sig_o, ao[:], OP.mult, OP.add)
                        qeff_c += [qe_e, qe_o]
                    for m in range(MT):
                        pb = pbp.tile([128, BLK], f32, name="psb")
                        for dti in range(DT):
                            mm(pb[:],
                               qeff_c[dti][:, m * 128:(m + 1) * 128],
                               keff_c[dti][:],
                               start=(dti == 0), stop=(dti == DT - 1))
                        ob = outp.tile([128, BLK], f32, name="ob", tag="ob")
                        nc.scalar.activation(ob[:], pb[:], AF.Copy)
                        nc.sync.dma_start(
                            OUT[c, m * 128:(m + 1) * 128, cols], ob[:])

                keffs = {0: emit_a(0)}
                for c in range(C):
                    if c + 1 < C:
                        keffs[c + 1] = emit_a(c + 1)
                    emit_b(c, keffs.pop(c))

        if reps and reps > 1:
            with tc.For_i(0, reps, 1):
                emit_body()
        else:
            emit_body()

    nc.compile()
    return nc


_PROG_CACHE = {}


def kernel(**inputs):
    from concourse.bass_utils import run_bass_kernel_spmd

    in_maps, metas = _host_prep(**inputs)
    if "prog" not in _PROG_CACHE:
        _PROG_CACHE["prog"] = _build_program()
    nc = _PROG_CACHE["prog"]

    res = run_bass_kernel_spmd(nc, in_maps, list(range(N_CORES)))
    out = np.empty((B, S, S, C), np.float32)
    for core in range(N_CORES):
        b, qt = metas[core]["b"], metas[core]["qt"]
        o = res.results[core]["OUT"]  # [C, RPC, S]
        out[b, qt * RPC:(qt + 1) * RPC] = o.transpose(1, 2, 0)
    return out



# revision 2
# speedup vs baseline: 1.2999x; 1.2999x over previous
"""Trainium2 Bass kernel for nn_BertWordPair (sparse_attention).

Computes: y = x @ W1 + b1 -> split into (q_tok, q_utt, k_tok, k_utt) per
channel c in [0,3); RoPE with block-sign structure from seg_ids; output
logits [B, S, S, 3] = sum over the two groups of the selected-variant
bilinear forms.

Strategy (8 NeuronCores), v2:
  - 2x2 output tiling per batch: core = (b, row-half, col-half); each core
    owns a 1024 x 1024 tile of the S x S logits for one batch.  Dense1 is
    computed for the core's 1024 rows (Q side) and 1024 cols (K side):
    halves the K-side dense duplication vs row-quarter sharding
    (245,760 PE cycles/core vs 282,624).
  - All matmuls in bf16 (full PE rate, half the DMA/SBUF of f32), PSUM
    accumulate in f32.  Output written as bf16 and upcast on host.
  - Engine balance: the psum+bias -> bf16 step runs on the Scalar
    (Activation) engine; the four RoPE products per feature pair are plain
    bf16 tensor-tensor multiplies on DVE (2-byte fast mode); the per-block
    sign selection (q_pos/q_neg, k_pos/k_neg from seg_ids variants) is a
    data-driven scalar_tensor_tensor with host-provided +-1 scalars, so the
    program stays SPMD-uniform while each core applies its own signs.
  - RoPE rotate-half never crosses partitions: host permutes W1 columns so
    even/odd features live in separate 128-partition tiles (pair (2k,2k+1)
    at partition k of the even/odd tile).
"""
import sys
sys.path.insert(0, '/opt/trn_rl_repo')

import numpy as np
import ml_dtypes

BF16 = ml_dtypes.bfloat16

B, S, H, C = 2, 2048, 768, 3
DG = 256             # rope dim per group (tok / utt)
N_CORES = 8
RR = 1024            # rows per core
CC = 1024            # cols per core
BLK = 512            # stage-B column block == seg length
NBK = CC // BLK      # 2 col blocks per core
KH = H // 128        # 6 contraction tiles for dense1
FT = 12              # feature tiles per side (q or k): 3 ch x 2 grp x 2 par
NPAIR = FT // 2      # 6 rope (even,odd) pairs per side
MT = BLK // 128      # 4 row tiles per 512-row half


def _variant(s, t):
    # 0=PP, 1=NP (q_neg*k_pos), 2=PN (q_pos*k_neg)
    if s >= 1 and t > s:
        return 1
    if t >= 1 and s > t:
        return 2
    return 0


def _rope_tables_half(pos, base):
    """pos: [n] ints -> cos [128, n], sin [128, n]; row k = freq k."""
    freq = np.power(float(base), -2.0 * np.arange(DG // 2, dtype=np.float64) / DG)
    ang = freq[:, None] * pos[None, :].astype(np.float64)
    return np.cos(ang).astype(np.float32), np.sin(ang).astype(np.float32)


def _perm_cols(side_off):
    """New feature order: c*512 + g*256 + p*128 + k  <-  orig
    c*1024 + side_off + g*256 + 2k + p."""
    cols = np.empty(C * 512, np.int64)
    f = 0
    for c in range(C):
        for g in range(2):
            for p in range(2):
                base = c * 1024 + side_off + g * 256 + p
                cols[f:f + 128] = base + 2 * np.arange(128)
                f += 128
    return cols


def _pack_pmajor(a, nt):
    """[nt*128, F] -> [128, nt, F] (partition-major chunks)."""
    F = a.shape[1]
    return np.ascontiguousarray(a.reshape(nt, 128, F).transpose(1, 0, 2))


def _tables(tok_pos, utt_pos):
    """-> [128, 2(cos/sin), 2(tok/utt), n] bf16."""
    ct, st = _rope_tables_half(tok_pos, 10000.0)
    cu, su = _rope_tables_half(utt_pos, 15.0)
    t = np.stack([np.stack([ct, cu], 0), np.stack([st, su], 0)], 0)
    return np.ascontiguousarray(t.transpose(2, 0, 1, 3)).astype(BF16)


def _host_prep(x, W1, b1, token_index, utterance_index, seg_ids):
    """Build per-core input maps + check fast-path validity."""
    x = np.asarray(x, np.float32)
    W1 = np.asarray(W1, np.float32)
    b1 = np.asarray(b1, np.float32)
    token_index = np.asarray(token_index)
    utterance_index = np.asarray(utterance_index)
    seg_ids = np.asarray(seg_ids)

    qcols = _perm_cols(0)     # q_tok at +0, q_utt at +256
    kcols = _perm_cols(512)   # k_tok at +512, k_utt at +768
    WQp = _pack_pmajor(np.ascontiguousarray(W1[:, qcols]), KH).astype(BF16)
    WKp = _pack_pmajor(np.ascontiguousarray(W1[:, kcols]), KH).astype(BF16)
    bQ = b1[qcols].astype(np.float32)
    bK = b1[kcols].astype(np.float32)
    biasc = np.ascontiguousarray(
        np.concatenate([bQ, bK]).reshape(2 * FT, 128).T)  # [128, 24] f32

    xTp = [_pack_pmajor(np.ascontiguousarray(x[b].T), KH).astype(BF16)
           for b in range(B)]

    # seg must be uniform per 512-chunk for the fast path
    segq = np.empty((B, 4), np.int64)
    for b in range(B):
        sb = seg_ids[b].reshape(4, BLK)
        if not np.all(sb == sb[:, :1]):
            raise NotImplementedError("fast path: seg must be uniform per 512-chunk")
        segq[b] = sb[:, 0]

    in_maps, metas = [], []
    for core in range(N_CORES):
        b, rh, ch = core // 4, (core // 2) % 2, core % 2
        rows = slice(rh * RR, (rh + 1) * RR)
        cols = slice(ch * CC, (ch + 1) * CC)

        TABQ = _tables(token_index[b, rows], utterance_index[b, rows])
        TABK = _tables(token_index[b, cols], utterance_index[b, cols])

        # sign scalars per (slot, row-half h, col block nb)
        SIGQ = np.empty((2, 2, NBK), np.float32)
        SIGK = np.empty((2, 2, NBK), np.float32)
        for h in range(2):
            s = int(segq[b, rh * 2 + h])
            for nb in range(NBK):
                t = int(segq[b, ch * 2 + nb])
                v = _variant(s, t)
                sq = -1.0 if v == 1 else 1.0
                sk = -1.0 if v == 2 else 1.0
                SIGQ[:, h, nb] = (-sq, sq)
                SIGK[:, h, nb] = (-sk, sk)
        SIGQ = np.ascontiguousarray(
            np.broadcast_to(SIGQ[None], (128, 2, 2, NBK)).copy())
        SIGK = np.ascontiguousarray(
            np.broadcast_to(SIGK[None], (128, 2, 2, NBK)).copy())

        in_maps.append({
            "XR": np.ascontiguousarray(xTp[b][:, :, rows]),
            "XC": np.ascontiguousarray(xTp[b][:, :, cols]),
            "WQ": WQp, "WK": WKp, "BIASC": biasc,
            "TABQ": TABQ, "TABK": TABK, "SIGQ": SIGQ, "SIGK": SIGK,
        })
        metas.append({"b": b, "rh": rh, "ch": ch})
    return in_maps, metas


def _build_program(reps=0):
    """Build the SPMD-uniform Bass program."""
    import concourse.bacc as bacc
    import concourse.mybir as mybir
    import concourse.tile as tile
    from contextlib import ExitStack

    f32 = mybir.dt.float32
    bf16 = mybir.dt.bfloat16
    AF = mybir.ActivationFunctionType
    OP = mybir.AluOpType

    nc = bacc.Bacc("TRN2", target_bir_lowering=False, debug=False,
                   num_devices=N_CORES)
    XRd = nc.dram_tensor("XR", [128, KH, RR], bf16, kind="ExternalInput")
    XCd = nc.dram_tensor("XC", [128, KH, CC], bf16, kind="ExternalInput")
    WQd = nc.dram_tensor("WQ", [128, KH, C * 512], bf16, kind="ExternalInput")
    WKd = nc.dram_tensor("WK", [128, KH, C * 512], bf16, kind="ExternalInput")
    BIASC = nc.dram_tensor("BIASC", [128, 2 * FT], f32, kind="ExternalInput")
    TABQd = nc.dram_tensor("TABQ", [128, 2, 2, RR], bf16, kind="ExternalInput")
    TABKd = nc.dram_tensor("TABK", [128, 2, 2, CC], bf16, kind="ExternalInput")
    SIGQd = nc.dram_tensor("SIGQ", [128, 2, 2, NBK], f32, kind="ExternalInput")
    SIGKd = nc.dram_tensor("SIGK", [128, 2, 2, NBK], f32, kind="ExternalInput")
    OUT = nc.dram_tensor("OUT", [C, RR, CC], bf16, kind="ExternalOutput")

    with tile.TileContext(nc) as tc, ExitStack() as ctx:
        biasp = ctx.enter_context(tc.tile_pool(name="biasp", bufs=1))
        tabp = ctx.enter_context(tc.tile_pool(name="tabp", bufs=1))
        xp = ctx.enter_context(tc.tile_pool(name="xp", bufs=2))
        wp = ctx.enter_context(tc.tile_pool(name="wp", bufs=3))
        fp = ctx.enter_context(tc.tile_pool(name="fp", bufs=4))
        aqp = ctx.enter_context(tc.tile_pool(name="aqp", bufs=48))
        akp = ctx.enter_context(tc.tile_pool(name="akp", bufs=30))
        effp = ctx.enter_context(tc.tile_pool(name="effp", bufs=32))
        outp = ctx.enter_context(tc.tile_pool(name="outp", bufs=4))
        pap = ctx.enter_context(tc.tile_pool(name="pap", bufs=4, space="PSUM"))
        pbp = ctx.enter_context(tc.tile_pool(name="pbp", bufs=3, space="PSUM"))

        mm = nc.tensor.matmul

        def emit_body():
            bias_all = biasp.tile([128, 2 * FT], f32, name="bias_all")
            nc.sync.dma_start(bias_all[:], BIASC[:])
            sigq = biasp.tile([128, 2, 2, NBK], f32, name="sigq")
            nc.sync.dma_start(sigq[:], SIGQd[:])
            sigk = biasp.tile([128, 2, 2, NBK], f32, name="sigk")
            nc.sync.dma_start(sigk[:], SIGKd[:])

            # ---- loads: xr + wq interleaved per-kh so PE starts early ----
            xr = xp.tile([128, KH, RR], bf16, name="xr", tag="x")
            wq_t = [wp.tile([128, KH, 512], bf16, name="wt", tag="wt")
                    for _ in range(3)]
            for kh in range(KH):
                nc.sync.dma_start(xr[:, kh, :], XRd[:, kh, :])
                for third in range(3):
                    nc.sync.dma_start(
                        wq_t[third][:, kh, :],
                        WQd[:, kh, third * 512:(third + 1) * 512])
            tabq = tabp.tile([128, 2, 2, RR], bf16, name="tabq")
            nc.sync.dma_start(tabq[:], TABQd[:])

            def stage_a(w_parts, xtile, ft, xoff, psum):
                third, fo = divmod(ft, 4)
                for kh in range(KH):
                    mm(psum[:],
                       w_parts[third][:, kh, fo * 128:(fo + 1) * 128],
                       xtile[:, kh, xoff:xoff + BLK],
                       start=(kh == 0), stop=(kh == KH - 1))

            def products(w_parts, xtile, tab, side_off, pr, half, pool):
                """dense pair + bias + 4 rope products -> (ae, as_, ao, bo)."""
                f_e = 2 * pr
                ps_e = pap.tile([128, BLK], f32, name="psa", tag="psa")
                stage_a(w_parts, xtile, f_e, half * BLK, ps_e)
                ps_o = pap.tile([128, BLK], f32, name="psa", tag="psa")
                stage_a(w_parts, xtile, f_e + 1, half * BLK, ps_o)
                fe = fp.tile([128, BLK], bf16, name="fe", tag="f")
                nc.scalar.activation(fe[:], ps_e[:], AF.Identity,
                                     bias=bias_all[:, side_off + f_e:side_off + f_e + 1])
                fo = fp.tile([128, BLK], bf16, name="fo", tag="f")
                nc.scalar.activation(fo[:], ps_o[:], AF.Identity,
                                     bias=bias_all[:, side_off + f_e + 1:side_off + f_e + 2])
                g = pr % 2
                cos = tab[:, 0, g, half * BLK:(half + 1) * BLK]
                sin = tab[:, 1, g, half * BLK:(half + 1) * BLK]
                ae = pool.tile([128, BLK], bf16, name="ae", tag="ab")
                nc.vector.tensor_mul(ae[:], fe[:], cos)
                bo = pool.tile([128, BLK], bf16, name="bo", tag="ab")
                nc.vector.tensor_mul(bo[:], fe[:], sin)
                as_ = pool.tile([128, BLK], bf16, name="as_", tag="ab")
                nc.vector.tensor_mul(as_[:], fo[:], sin)
                ao = pool.tile([128, BLK], bf16, name="ao", tag="ab")
                nc.vector.tensor_mul(ao[:], fo[:], cos)
                return ae, as_, ao, bo

            # ---------- Q side: dense + products for both row halves ----------
            abq = {}
            for pr in range(NPAIR):
                for h in range(2):
                    abq[pr, h] = products(wq_t, xr, tabq, 0, pr, h, aqp)

            # ---- loads for K side (stream during Q compute) ----
            xc = xp.tile([128, KH, CC], bf16, name="xc", tag="x")
            nc.sync.dma_start(xc[:], XCd[:])
            wk_t = [wp.tile([128, KH, 512], bf16, name="wt", tag="wt")
                    for _ in range(3)]
            for third in range(3):
                nc.sync.dma_start(wk_t[third][:], WKd[:, :, third * 512:(third + 1) * 512])
            tabk = tabp.tile([128, 2, 2, CC], bf16, name="tabk")
            nc.sync.dma_start(tabk[:], TABKd[:])

            def emit_k(nb):
                return {pr: products(wk_t, xc, tabk, FT, pr, nb, akp)
                        for pr in range(NPAIR)}

            def emit_b(nb, abk_nb):
                for h in range(2):
                    sq_e = sigq[:, 0, h, nb:nb + 1]
                    sq_o = sigq[:, 1, h, nb:nb + 1]
                    sk_e = sigk[:, 0, h, nb:nb + 1]
                    sk_o = sigk[:, 1, h, nb:nb + 1]
                    for c in range(C):
                        qeff, keff = [], []
                        for g in range(2):
                            pr = c * 2 + g
                            ae, as_, ao, bo = abq[pr, h]
                            qe = effp.tile([128, BLK], bf16, name="qe", tag="eff")
                            nc.vector.scalar_tensor_tensor(
                                qe[:], as_[:], sq_e, ae[:], OP.mult, OP.add)
                            qo = effp.tile([128, BLK], bf16, name="qo", tag="eff")
                            nc.vector.scalar_tensor_tensor(
                                qo[:], bo[:], sq_o, ao[:], OP.mult, OP.add)
                            ae, as_, ao, bo = abk_nb[pr]
                            ke = effp.tile([128, BLK], bf16, name="ke", tag="eff")
                            nc.vector.scalar_tensor_tensor(
                                ke[:], as_[:], sk_e, ae[:], OP.mult, OP.add)
                            ko = effp.tile([128, BLK], bf16, name="ko", tag="eff")
                            nc.vector.scalar_tensor_tensor(
                                ko[:], bo[:], sk_o, ao[:], OP.mult, OP.add)
                            qeff += [qe, qo]
                            keff += [ke, ko]
                        for m in range(MT):
                            pb = pbp.tile([128, BLK], f32, name="psb", tag="psb")
                            for dti in range(4):
                                mm(pb[:],
                                   qeff[dti][:, m * 128:(m + 1) * 128],
                                   keff[dti][:],
                                   start=(dti == 0), stop=(dti == 3))
                            ob = outp.tile([128, BLK], bf16, name="ob", tag="ob")
                            nc.scalar.activation(ob[:], pb[:], AF.Identity)
                            nc.sync.dma_start(
                                OUT[c, h * BLK + m * 128:h * BLK + (m + 1) * 128,
                                    nb * BLK:(nb + 1) * BLK], ob[:])

            # K block 0, stage B block 0, K block 1, stage B block 1
            abk0 = emit_k(0)
            emit_b(0, abk0)
            abk1 = emit_k(1)
            emit_b(1, abk1)

        if reps and reps > 1:
            with tc.For_i(0, reps, 1):
                emit_body()
        else:
            emit_body()

    nc.compile()
    return nc


_PROG_CACHE = {}


def kernel(**inputs):
    from concourse.bass_utils import run_bass_kernel_spmd

    in_maps, metas = _host_prep(**inputs)
    if "prog" not in _PROG_CACHE:
        _PROG_CACHE["prog"] = _build_program()
    nc = _PROG_CACHE["prog"]

    res = run_bass_kernel_spmd(nc, in_maps, list(range(N_CORES)))
    out = np.empty((B, S, S, C), np.float32)
    for core in range(N_CORES):
        m = metas[core]
        o = np.asarray(res.results[core]["OUT"], np.float32)  # [C, RR, CC]
        out[m["b"], m["rh"] * RR:(m["rh"] + 1) * RR,
            m["ch"] * CC:(m["ch"] + 1) * CC] = o.transpose(1, 2, 0)
    return out
